# revision 3
# baseline (speedup 1.0000x reference)
"""Self-contained Trainium kernel for nn_BipartiteGNN_WMMSE_Layer.

B=256, K=32, N=64, HID=64. Pure data-parallel: batch is sharded 32-per-core
across the 8 NeuronCores (jax.pmap / PJRT); the tiny MLP weights are
replicated. All aggregations are within a batch element, so there is no
cross-device communication. Returns (B, N, K, 2) float32.
"""
import numpy as np

B, K, N, HID = 256, 32, 64, 64
NCORES = 8
NOISE_VAR = 1e-3
LN_EPS = 1e-5

_compiled = {}


def _get_fn():
    if "fn" in _compiled:
        return _compiled["fn"]
    import jax
    import jax.numpy as jnp

    def _layernorm(x, g, b):
        m = jnp.mean(x, axis=-1, keepdims=True)
        v = jnp.mean((x - m) ** 2, axis=-1, keepdims=True)
        return (x - m) * jax.lax.rsqrt(v + LN_EPS) * g + b

    def shard_fn(H_re, H_im, a_re, a_im, Wp_re, Wp_im,
                 W1, b1, g1, be1, W2, b2,
                 U1, ub1, ug1, ube1, U2, ub2, step):
        b, k, n = H_re.shape
        HW_re = jnp.matmul(H_re, Wp_re) - jnp.matmul(H_im, Wp_im)
        HW_im = jnp.matmul(H_re, Wp_im) + jnp.matmul(H_im, Wp_re)
        eye = jnp.eye(k, dtype=jnp.float32)
        sg_re = jnp.sum(HW_re * eye, axis=-1)            # diag, no gather
        sg_im = jnp.sum(HW_im * eye, axis=-1)
        p = jnp.sum(HW_re ** 2 + HW_im ** 2, axis=-1)
        rp = 1.0 / (p + NOISE_VAR)
        U_re = sg_re * rp
        U_im = sg_im * rp
        E = 1.0 - (U_re * sg_re + U_im * sg_im)
        w = 1.0 / jnp.maximum(E, 1e-6)

        WpT_re = jnp.swapaxes(Wp_re, 1, 2)
        WpT_im = jnp.swapaxes(Wp_im, 1, 2)
        # Z @ W1 decomposed: per-edge terms + per-user + per-antenna
        zw = (H_re[..., None] * W1[0] + H_im[..., None] * W1[1]
              + WpT_re[..., None] * W1[2] + WpT_im[..., None] * W1[3])
        user_in = jnp.stack([U_re, U_im, w], axis=-1)    # (b,K,3)
        ant_in = jnp.stack([a_re, a_im], axis=-1)        # (b,N,2)
        zw = zw + (user_in @ W1[4:7])[:, :, None, :]
        zw = zw + (ant_in @ W1[7:9])[:, None, :, :]
        zw = zw + b1

        h = _layernorm(zw, g1, be1)
        h = jax.nn.relu(h)
        E_feat = jax.nn.relu(h @ W2 + b2)                # (b,K,N,H)

        user_feat = jnp.mean(E_feat, axis=2)             # (b,K,H)
        ant_feat = jnp.mean(E_feat, axis=1)              # (b,N,H)

        u = E_feat.reshape(b, k * n, HID) @ U1[:HID]
        u = u.reshape(b, k, n, HID)
        u = u + (user_feat @ U1[HID:2 * HID])[:, :, None, :]
        u = u + (ant_feat @ U1[2 * HID:])[:, None, :, :]
        u = u + ub1
        u = _layernorm(u, ug1, ube1)
        u = jax.nn.relu(u)
        delta = u.reshape(b, k * n, HID) @ U2 + ub2
        delta = delta.reshape(b, k, n, 2)

        dW = jnp.swapaxes(delta, 1, 2)                   # (b,N,K,2)
        Wn_re = Wp_re + step * dW[..., 0]
        Wn_im = Wp_im + step * dW[..., 1]
        return jnp.stack([Wn_re, Wn_im], axis=-1)

    fn = jax.pmap(
        shard_fn,
        in_axes=(0, 0, 0, 0, 0, 0,
                 None, None, None, None, None, None,
                 None, None, None, None, None, None, None),
        devices=jax.devices()[:NCORES],
    )
    _compiled["fn"] = fn
    return fn


def kernel(H_re, H_im, a_re, a_im, Wp_re, Wp_im,
           W1, b1, g1, be1, W2, b2,
           U1, ub1, ug1, ube1, U2, ub2, step, **_unused):
    f32 = np.float32
    shard = lambda x: np.asarray(x, f32).reshape(NCORES, B // NCORES, *np.shape(x)[1:])
    H_re_s, H_im_s = shard(H_re), shard(H_im)
    a_re_s, a_im_s = shard(a_re), shard(a_im)
    Wp_re_s, Wp_im_s = shard(Wp_re), shard(Wp_im)
    args = (np.asarray(W1, f32), np.asarray(b1, f32), np.asarray(g1, f32),
            np.asarray(be1, f32), np.asarray(W2, f32), np.asarray(b2, f32),
            np.asarray(U1, f32), np.asarray(ub1, f32), np.asarray(ug1, f32),
            np.asarray(ube1, f32), np.asarray(U2, f32), np.asarray(ub2, f32),
            np.asarray(step, f32))
    fn = _get_fn()
    out = fn(H_re_s, H_im_s, a_re_s, a_im_s, Wp_re_s, Wp_im_s, *args)
    out = np.asarray(out)                                # (8, 32, N, K, 2)
    return out.reshape(B, N, K, 2)
